# revision 35
# baseline (speedup 1.0000x reference)
"""Causal self-attention (B=8, T=1024, D=2048, H=16) on 8 NeuronCores.

Data-parallel over the batch dim: core i handles batch element i end-to-end
(QKV proj -> causal attention -> out proj). No collectives.

Layout: everything runs on transposed activations. The host feeds x[b].T
partition-packed ([128, 16*1024] fp16); Q/K are produced d-major ([Dh, T]),
V token-major, and the output projection emits y.T which the host transposes
back. All matmul operands are fp16 (same PE rate as f32r but half the
LdWeights/DMA/SBUF traffic); PSUM accumulation is fp32.

DMA: per-queue throughput is descriptor-rate limited (~2KB partition-row
descriptors move only ~70-90 GB/s), so every bulk input is host-packed such
that each DMA has >=4KB contiguous per partition: xT in 2/4-c-tile chunks
(4-8KB runs), V-weights in quads of [128,512] tiles (4KB runs), Q/K weights
per (q/k, head) with both contraction halves fused (4KB runs). x rides the
sync queue, weights ride scalar; a short chain of dummy matmuls on a memset
region warms the PE clock (HAM un-throttle) while the first tiles fly.

Softmax skips the max-subtraction (scores are ~N(0,1); exp is safely in fp16
range). Exp tiles are summed on the DVE and one ones-matrix matmul per
q-chunk then yields the softmax denominator already broadcast across
partitions (a gpsimd partition_all_reduce was tried instead: at ~2.1us/op it
backpressures the chunk pipeline -- keep it on the PE). Causal structure is
exploited at q-chunk=256 granularity (k-tiles 0..2jc+1 per chunk) with the
mask applied as a DVE multiply against precomputed 0/1 tiles. Q/K
projections run as four half-chains interleaved with the attention chunks
that consume them. Per-head attention outputs and a row-contiguous copy of
w_proj stay resident in SBUF, so the output projection starts with
everything on-chip. The first out-proj accumulators borrow the idle Q/K
PSUM banks and the last output tile drains in 256-col chunks to shorten the
tail.
"""

import math

import numpy as np

B, T, D = 8, 1024, 2048
H = 16
DH = D // H  # 128
NCT = D // 128  # 16 c-tiles
QC = 256  # q-chunk for causal attention
NQC = T // QC  # 4
SCALE = 1.0 / math.sqrt(DH)
N_CORES = 8

_CACHE = {}


def _build():
    import concourse.bacc as bacc
    import concourse.mybir as mybir
    import concourse.tile as tile

    f32 = mybir.dt.float32
    f16 = mybir.dt.float16
    Exp = mybir.ActivationFunctionType.Exp

    nc = bacc.Bacc(None, target_bir_lowering=False)

    # x.T partition-packed: [p, ct*1024 + t] = x[t, ct*128 + p]
    xT = nc.declare_dram_parameter("xT", [128, NCT * T], f16, isOutput=False)
    # V weights partition-packed in (fc, quad) blocks: column
    # fc*8192 + ct*512 + j of partition p = w_qkv[ct*128 + p, 2D + fc*512 + j]
    w_vp = nc.declare_dram_parameter("w_vp", [128, 4 * NCT * 512], f16, isOutput=False)
    b_v = nc.declare_dram_parameter("b_v", [D], f16, isOutput=False)
    w_proj = nc.declare_dram_parameter("w_proj", [D, D], f16, isOutput=False)
    # biases host-packed p-major: col n<48 = b_qkv[n*128+p], col 48+n = b_proj[n*128+p]
    bias_pk = nc.declare_dram_parameter("bias_pk", [128, 64], f32, isOutput=False)
    # causal masks (keep where k <= q) for the two diagonal k-tiles, plus a
    # ones block for the softmax-denominator matmul
    consts = nc.declare_dram_parameter(
        "consts", [128, 2 * QC + 128], f16, isOutput=False
    )
    # Q/K weights partition-packed per (s, h): [p, (s*H+h)*2048 + hf*1024 +
    # n*128 + f] = w_qkv[hf*1024 + n*128 + p, s*D + h*128 + f]
    w_qkp = nc.declare_dram_parameter("w_qkp", [128, 2 * H * 2048], f16, isOutput=False)
    outT = nc.declare_dram_parameter("outT", [D, T], f32, isOutput=True)

    with tile.TileContext(nc) as tc:
        with (
            tc.tile_pool(name="xbig", bufs=1) as pool_xbig,
            tc.tile_pool(name="vbig", bufs=1) as pool_vbig,
            tc.tile_pool(name="aobig", bufs=1) as pool_aobig,
            tc.tile_pool(name="qk", bufs=4) as pool_qk,
            tc.tile_pool(name="e", bufs=4) as pool_e,
            tc.tile_pool(name="esum", bufs=2) as pool_esum,
            tc.tile_pool(name="wq", bufs=2) as pool_wq,
            tc.tile_pool(name="wbig", bufs=2) as pool_wbig,
            tc.tile_pool(name="wproj", bufs=2) as pool_wproj,
            tc.tile_pool(name="outp", bufs=2) as pool_out,
            tc.tile_pool(name="den", bufs=2) as pool_den,
            tc.tile_pool(name="misc", bufs=1) as pool_misc,
        ):
            pool_qa_cm = tc.tile_pool(name="qaps", bufs=2, space="PSUM")
            pool_qa = pool_qa_cm.__enter__()

            # ---- PE warmup: a memset region + 6 dummy N=512 matmuls issued
            # before any data lands releases the HAM clock throttle (~3.4us
            # of sustained PE activity) while the first DMAs are in flight,
            # so the first real matmuls run at 2.4 GHz instead of 1.2. The
            # scratch operand borrows V_sb (first written ~30us in). ----
            V_sb = pool_vbig.tile([128, T // 128, D], f16, tag="vbig")
            warm_sb = V_sb[:, 0, 0:512]
            nc.gpsimd.memset(warm_sb, 0.0)
            warm_ps = pool_qa.tile([128, 512], f32, name="warm_ps", tag="qa")
            for _ in range(12):
                nc.tensor.matmul(
                    warm_ps[:], warm_sb[:, 0:128], warm_sb, start=True, stop=True
                )

            # ---- load x.T resident: [128, 16, 1024], one slot per c-tile.
            # Chunked 2/4-tile DMAs (4-8KB contiguous per partition) run at
            # byte rate on the sync queue; fc=0 V-weight quads ride scalar. ----
            xT_all = pool_xbig.tile([128, NCT, T], f16, tag="xbig")
            xT_t = [xT_all[:, ct, :] for ct in range(NCT)]

            def dma_x(c0, c1, eng=None):
                (eng or nc.sync).dma_start(
                    xT_all[:, c0:c1, :], xT[:, c0 * T : c1 * T]
                )

            def dma_wq(fc, q, eng=None, split=False):
                # one quad = c-tiles 4q..4q+3 of feature chunk fc
                w_t = pool_wq.tile([128, 4, 512], f16, name="w_t", tag="wq")
                base = fc * 8192 + q * 2048
                eng = eng or nc.scalar
                if split:
                    # two half-quad DMAs so the first pair lands sooner
                    eng.dma_start(w_t[:, 0:2, :], w_vp[:, base : base + 1024])
                    eng.dma_start(w_t[:, 2:4, :], w_vp[:, base + 1024 : base + 2048])
                else:
                    eng.dma_start(w_t[:], w_vp[:, base : base + 2048])
                return w_t

            # Runs are capped at 4KB per partition: anything bigger (the
            # 8KB-run variant) trips a chip-wide ~0.83x power downclock (P0)
            # that sticks for the whole kernel -- see the v3 trace.
            # Earliest-deadline order on two ~115GB/s queues: ALL x on sync
            # (smallest pieces first so the first chain starts ~11us in),
            # ALL weights + smalls on scalar. The gpsimd software DGE
            # measured <~13GB/s -- useless for bulk.
            dma_x(0, 1)
            wq0 = dma_wq(0, 0, split=True)
            dma_x(1, 2)
            wq1 = dma_wq(0, 1)
            dma_x(2, 4)
            dma_x(4, 6)
            dma_x(6, 8)
            dma_x(8, 10)
            dma_x(10, 12)
            wq2 = dma_wq(0, 2)
            wq3 = dma_wq(0, 3)
            # the x tail rides scalar (3MB per queue, balanced): sync alone
            # would run ~4us behind consumption by c-tile 12, and that
            # stall also re-throttles the HAM clock
            dma_x(12, 14, nc.scalar)
            dma_x(14, 16, nc.scalar)

            # ---- constants / biases: contiguous host-packed, cheap ----
            cpack = pool_misc.tile([128, 2 * QC + 128], f16, tag="cpack")
            nc.scalar.dma_start(cpack[:], consts[:])
            masks = [cpack[:, 0:QC], cpack[:, QC : 2 * QC]]
            ones_blk = cpack[:, 2 * QC : 2 * QC + 128]
            bias_sb = pool_misc.tile([128, 64], f32, tag="biaspk")
            nc.scalar.dma_start(bias_sb[:], bias_pk[:])
            bqkv_sb = bias_sb[:, 0:48]
            bproj_sb = bias_sb[:, 48:64]
            # V-bias broadcast to all partitions once on gpsimd (DMA lands
            # in partition 0, broadcast fills the rest in place); the V
            # PSUM->SBUF copy then fuses the add on the DVE
            bv_bcast = pool_misc.tile([128, D], f16, tag="bv_bcast")
            nc.scalar.dma_start(
                bv_bcast[0:1, :], b_v[:].rearrange("(o f) -> o f", o=1)
            )
            nc.gpsimd.partition_broadcast(bv_bcast[:, :], bv_bcast[0:1, :])

            # ---- phase 1: V for all heads, token-major [128, 8, 2048].
            # Six PSUM banks (token tiles in a 6+2 sub-pass split, weights
            # stay resident across both) so phase 2's Q/K accumulators are
            # pre-allocated and never wait on the phase-1 drain. ----
            with tc.tile_pool(name="p1psum", bufs=6, space="PSUM") as pool_p1:
                for fc in range(D // 512):
                    if fc == 0:
                        w_qs = [wq0, wq1, wq2, wq3]
                    else:
                        w_qs = [dma_wq(fc, q) for q in range(4)]
                    # token tiles 0-5 use the phase-1 banks; tiles 6-7
                    # borrow the (idle) phase-2 Q/K accumulator banks
                    ps_v = [
                        pool_p1.tile([128, 512], f32, name="vps", tag="vps")
                        for _ in range(6)
                    ] + [
                        pool_qa.tile([128, 512], f32, name="vps_qa", tag="qa")
                        for _ in range(2)
                    ]
                    for ct in range(NCT):
                        # tt order matches the drain order below, so the
                        # next fc's first matmul waits only on the first
                        # drained bank instead of the third
                        for tt in (6, 7, 0, 1, 2, 3, 4, 5):
                            nc.tensor.matmul(
                                ps_v[tt][:],
                                xT_t[ct][:, tt * 128 : (tt + 1) * 128],
                                w_qs[ct // 4][:, ct % 4, :],
                                start=(ct == 0),
                                stop=(ct == NCT - 1),
                            )
                    # tiles 6-7 first: they hold the borrowed Q/K banks,
                    # so the first phase-2 chain isn't stuck behind the
                    # whole serial DVE copy drain
                    for tt in (6, 7, 0, 1, 2, 3, 4, 5):
                        # fused += b_v during the PSUM->SBUF copy
                        nc.vector.tensor_add(
                            V_sb[:, tt, fc * 512 : (fc + 1) * 512],
                            ps_v[tt][:],
                            bv_bcast[:, fc * 512 : (fc + 1) * 512],
                        )

            # ---- phase 2: per-head attention. Q/K projections run as four
            # half-chains (Q-half then K-half, bias-adds overlapping the
            # next chain); attention q-chunks 0-1 only need the first halves
            # of Q and K, so they interleave between the half-chains and the
            # PE never waits on a DVE bias-add. w_proj is staged into SBUF
            # row-contiguous (one c-tile per head) so phase 3 starts with
            # all weights resident. ----
            def emit_qk_weights(h, eng=None):
                eng = eng or nc.sync
                w_hs = {}
                for si, s in enumerate(("q", "k")):
                    w_t = pool_wbig.tile(
                        [128, 2, NCT // 2, 128], f16, name="w_t", tag="wbig", bufs=3
                    )
                    base = (si * H + h) * 2048
                    eng.dma_start(w_t[:], w_qkp[:, base : base + 2048])
                    w_hs[s] = [w_t[:, 0], w_t[:, 1]]
                return w_hs

            def emit_qk_half(h, s, w_halves, sb, jc):
                btile = h if s == "q" else NCT + h
                ps = pool_qa.tile([128, 512], f32, name="qkps", tag="qa")
                for ct in range(NCT):
                    nc.tensor.matmul(
                        ps[:],
                        w_halves[ct // 8][:, ct % 8, :],
                        xT_t[ct][:, jc * 512 : (jc + 1) * 512],
                        start=(ct == 0),
                        stop=(ct == NCT - 1),
                    )
                nc.vector.tensor_scalar_add(
                    sb[:, jc * 512 : (jc + 1) * 512],
                    ps[:],
                    bqkv_sb[:, btile : btile + 1],
                )

            def emit_attn_chunks(h, qk, ao_t, jcs):
                # causal attention, scores transposed [k, q],
                # q-chunks of 256 (k-tiles 0..2jc+1; rest masked)
                for jc in jcs:
                    nk = 2 * jc + 2
                    ps_y = pool_y.tile([128, QC], f32, tag="y")
                    e_sum = pool_esum.tile([128, QC], f16, tag="esum", bufs=3)
                    # k-tiles 0..nk-2 run full-width; the final diagonal
                    # k-tile's first 128 q-columns are fully masked, so it
                    # runs at half width into the upper half of the chunk
                    for ki in range(nk - 1):
                        ps_s = pool_s.tile([128, QC], f32, tag="mm256")
                        nc.tensor.matmul(
                            ps_s[:],
                            qk["k"][:, ki * 128 : (ki + 1) * 128],
                            qk["q"][:, jc * QC : (jc + 1) * QC],
                            start=True,
                            stop=True,
                        )
                        # exp of the first k-tile lands directly in e_sum
                        e_t = (
                            e_sum
                            if ki == 0
                            else pool_e.tile([128, QC], f16, tag="e", bufs=4)
                        )
                        nc.scalar.activation(e_t[:], ps_s[:], Exp, scale=SCALE)
                        if ki == 2 * jc:
                            # diagonal tile: keep where k <= q
                            nc.vector.tensor_mul(e_t[:], e_t[:], masks[0])
                        nc.tensor.matmul(
                            ps_y[:],
                            V_sb[:, ki, h * 128 : (h + 1) * 128],
                            e_t[:],
                            start=(ki == 0),
                            stop=(ki == nk - 2),
                        )
                        if ki > 0:
                            nc.vector.tensor_add(e_sum[:], e_sum[:], e_t[:])
                    ki = nk - 1
                    ps_s2 = pool_s.tile([128, 128], f32, name="ps_s2", tag="mm256")
                    nc.tensor.matmul(
                        ps_s2[:],
                        qk["k"][:, ki * 128 : (ki + 1) * 128],
                        qk["q"][:, jc * QC + 128 : (jc + 1) * QC],
                        start=True,
                        stop=True,
                    )
                    e_h = pool_e.tile([128, 128], f16, name="e_h", tag="eh", bufs=3)
                    nc.scalar.activation(e_h[:], ps_s2[:], Exp, scale=SCALE)
                    # within this half, keep where (f-128) >= p: the same
                    # triangle as mask0's first 128 columns
                    nc.vector.tensor_mul(e_h[:], e_h[:], masks[0][:, 0:128])
                    nc.tensor.matmul(
                        ps_y[:, 128:QC],
                        V_sb[:, ki, h * 128 : (h + 1) * 128],
                        e_h[:],
                        start=False,
                        stop=True,
                        skip_group_check=True,
                    )
                    nc.vector.tensor_add(
                        e_sum[:, 128:QC], e_sum[:, 128:QC], e_h[:]
                    )
                    # one ones-matrix matmul yields the denominator already
                    # broadcast across partitions: out[m, q] = sum_k e_sum[k, q]
                    ps_db = pool_s.tile([128, QC], f32, name="ps_db", tag="mm256")
                    nc.tensor.matmul(
                        ps_db[:], ones_blk, e_sum[:], start=True, stop=True
                    )
                    # approx reciprocal (~18 bits; denominators are
                    # bounded away from 0 by the diagonal exp term)
                    inv_b = pool_den.tile([128, QC], f32, name="inv_b", tag="invb")
                    nc.vector.reciprocal_approx_fast(out=inv_b[:], in_=ps_db[:])
                    nc.vector.tensor_mul(
                        ao_t[:, jc * QC : (jc + 1) * QC], ps_y[:], inv_b[:]
                    )

            with (
                tc.tile_pool(name="sps", bufs=4, space="PSUM") as pool_s,
                tc.tile_pool(name="yps", bufs=2, space="PSUM") as pool_y,
            ):
                ao_heads = []
                wp_full = []
                for h in range(H):
                    # stage one row-contiguous c-tile of w_proj per head on
                    # the scalar DMA queue (ready before phase 3 starts)
                    wp_t = pool_wproj.tile(
                        [128, D], f16, name="wp_t", tag="wproj", bufs=NCT
                    )
                    nc.scalar.dma_start(
                        wp_t[:], w_proj[h * 128 : (h + 1) * 128, :]
                    )
                    wp_full.append(wp_t)

                    # head 0's weights ride the scalar queue, which drains
                    # long before the sync queue's startup stream
                    w_hs = emit_qk_weights(h, nc.scalar if h == 0 else None)
                    q_sb = pool_qk.tile([128, T], f16, name="q_sb", tag="qk")
                    k_sb = pool_qk.tile([128, T], f16, name="k_sb", tag="qk")
                    qk = {"q": q_sb, "k": k_sb}
                    ao_t = pool_aobig.tile(
                        [128, T], f16, name="ao_t", tag="aobig", bufs=H
                    )
                    # chunks 0-1 read only the first halves of Q and K, so
                    # they run between the half-chains and hide the DVE
                    # bias-add latency
                    emit_qk_half(h, "q", w_hs["q"], q_sb, 0)
                    emit_qk_half(h, "k", w_hs["k"], k_sb, 0)
                    emit_qk_half(h, "q", w_hs["q"], q_sb, 1)
                    emit_attn_chunks(h, qk, ao_t, (0, 1))
                    emit_qk_half(h, "k", w_hs["k"], k_sb, 1)
                    emit_attn_chunks(h, qk, ao_t, (2, 3))
                    ao_heads.append(ao_t)

            # ---- phase 3: output projection, emitted transposed.
            # rhs for c-tile ct is exactly head ct's attention output
            # (f = h*128 + dh); weights and activations are all resident.
            # dt=0's accumulators come from the (already idle) Q/K banks so
            # the first chain isn't gated on the last attention drain; the
            # final dt drains in 256-col chunks to shorten the tail. ----
            with tc.tile_pool(name="p3psum", bufs=4, space="PSUM") as pool_p3:
                for dt in range(D // 128):
                    if dt == 0:
                        ps3 = [
                            pool_qa.tile([128, 512], f32, name="ps3qa", tag="qa")
                            for _ in range(2)
                        ]
                    else:
                        ps3 = [
                            pool_p3.tile([128, 512], f32, name="ps3", tag="mm512")
                            for _ in range(2)
                        ]
                    last = dt == D // 128 - 1

                    def drain(jc, c0, w):
                        o_t = pool_out.tile([128, w], f32, tag="outp", bufs=4)
                        nc.vector.tensor_scalar_add(
                            o_t[:], ps3[jc][:, c0 : c0 + w], bproj_sb[:, dt : dt + 1]
                        )
                        nc.sync.dma_start(
                            outT[
                                dt * 128 : (dt + 1) * 128,
                                jc * 512 + c0 : jc * 512 + c0 + w,
                            ],
                            o_t[:],
                        )

                    if not last:
                        for ct in range(NCT):
                            for jc in range(2):
                                nc.tensor.matmul(
                                    ps3[jc][:],
                                    wp_full[ct][:, dt * 128 : (dt + 1) * 128],
                                    ao_heads[ct][:, jc * 512 : (jc + 1) * 512],
                                    start=(ct == 0),
                                    stop=(ct == NCT - 1),
                                )
                        drain(0, 0, 512)
                        drain(1, 0, 512)
                    else:
                        # jc-sequential chains, jc1 as two N=256 chains on
                        # their own PSUM tiles (a shared tile serializes the
                        # second chain behind the first drain): every drain
                        # except the final 256-col one overlaps later
                        # matmuls, so the post-last-matmul pipe is one short
                        # add->DMA
                        for ct in range(NCT):
                            nc.tensor.matmul(
                                ps3[0][:],
                                wp_full[ct][:, dt * 128 : (dt + 1) * 128],
                                ao_heads[ct][:, 0:512],
                                start=(ct == 0),
                                stop=(ct == NCT - 1),
                            )
                        drain(0, 0, 512)
                        for half in range(2):
                            ps_h = pool_p3.tile(
                                [128, 256], f32, name="ps3h", tag="mm256c", bufs=2
                            )
                            for ct in range(NCT):
                                nc.tensor.matmul(
                                    ps_h[:],
                                    wp_full[ct][:, dt * 128 : (dt + 1) * 128],
                                    ao_heads[ct][
                                        :, 512 + half * 256 : 512 + half * 256 + 256
                                    ],
                                    start=(ct == 0),
                                    stop=(ct == NCT - 1),
                                )
                            o_t = pool_out.tile([128, 256], f32, tag="outp", bufs=4)
                            nc.vector.tensor_scalar_add(
                                o_t[:], ps_h[:], bproj_sb[:, dt : dt + 1]
                            )
                            # the very last piece goes out on the (idle)
                            # scalar queue so it isn't serialized behind the
                            # previous drains' transfers on sync
                            (nc.scalar if half == 1 else nc.sync).dma_start(
                                outT[
                                    dt * 128 : (dt + 1) * 128,
                                    512 + half * 256 : 768 + half * 256,
                                ],
                                o_t[:],
                            )

            pool_qa_cm.__exit__(None, None, None)

    nc.compile()
    return nc


def _get_nc():
    if "nc" not in _CACHE:
        _CACHE["nc"] = _build()
    return _CACHE["nc"]


def kernel(x, w_qkv, b_qkv, w_proj, b_proj, _trace=False, _trace_kwargs=None):
    from concourse.bass_utils import run_bass_kernel_spmd

    x = np.asarray(x, dtype=np.float32)
    w_qkv = np.asarray(w_qkv, dtype=np.float32)
    b_qkv = np.asarray(b_qkv, dtype=np.float32)
    w_proj = np.asarray(w_proj, dtype=np.float32)
    b_proj = np.asarray(b_proj, dtype=np.float32)

    w_qkv16 = np.ascontiguousarray(w_qkv.astype(np.float16))
    w_proj16 = np.ascontiguousarray(w_proj.astype(np.float16))
    b_v16 = np.ascontiguousarray(b_qkv[2 * D : 3 * D].astype(np.float16))

    # V weights partition-packed: [p, (fc, ct, j)] = w_v[ct*128+p, fc*512+j]
    w_vp = np.ascontiguousarray(
        w_qkv16[:, 2 * D : 3 * D]
        .reshape(NCT, 128, 4, 512)
        .transpose(1, 2, 0, 3)
        .reshape(128, 4 * NCT * 512)
    )

    # Q/K weights partition-packed: [p, (s, h, hf, n, f)] =
    # w_qkv[hf*1024 + n*128 + p, s*D + h*128 + f]
    w_qkp = np.ascontiguousarray(
        w_qkv16[:, : 2 * D]
        .reshape(2, 8, 128, 2, H, 128)
        .transpose(2, 3, 4, 0, 1, 5)
        .reshape(128, 2 * H * 2048)
    )

    # biases packed p-major so the device DMA is one contiguous copy
    bias_pk = np.concatenate(
        [
            b_qkv.reshape(3 * D // 128, 128).T,
            b_proj.reshape(D // 128, 128).T,
        ],
        axis=1,
    ).astype(np.float32)
    bias_pk = np.ascontiguousarray(bias_pk)

    # packed constants: causal masks (keep where k<=q) for the two diagonal
    # k-tiles, and a ones block for the denominator matmul
    consts = np.zeros((128, 2 * QC + 128), dtype=np.float16)
    p = np.arange(128)[:, None]
    f = np.arange(QC)[None, :]
    consts[:, 0:QC] = f >= p
    consts[:, QC : 2 * QC] = f >= p + 128
    consts[:, 2 * QC :] = 1.0

    nc = _get_nc()
    in_maps = []
    for i in range(N_CORES):
        xTi = np.ascontiguousarray(
            x[i]
            .astype(np.float16)
            .reshape(T, NCT, 128)
            .transpose(2, 1, 0)
            .reshape(128, NCT * T)
        )
        in_maps.append(
            {
                "xT": xTi,
                "w_vp": w_vp,
                "b_v": b_v16,
                "w_proj": w_proj16,
                "bias_pk": bias_pk,
                "consts": consts,
                "w_qkp": w_qkp,
            }
        )
    res = run_bass_kernel_spmd(
        nc,
        in_maps,
        list(range(N_CORES)),
        trace=_trace,
        **(_trace_kwargs or {}),
    )
    y = np.stack(
        [np.ascontiguousarray(res.results[i]["outT"].T) for i in range(N_CORES)]
    )
    if _trace:
        _CACHE["last_result"] = res
    return y


# revision 37
# speedup vs baseline: 1.0108x; 1.0108x over previous
"""Causal self-attention (B=8, T=1024, D=2048, H=16) on 8 NeuronCores.

Data-parallel over the batch dim: core i handles batch element i end-to-end
(QKV proj -> causal attention -> out proj). No collectives.

Layout: everything runs on transposed activations. The host feeds x[b].T
partition-packed ([128, 16*1024] fp16); Q/K are produced d-major ([Dh, T]),
V token-major, and the output projection emits y.T which the host transposes
back. All matmul operands are fp16 (same PE rate as f32r but half the
LdWeights/DMA/SBUF traffic); PSUM accumulation is fp32.

DMA: per-queue throughput is descriptor-rate limited (~2KB partition-row
descriptors move only ~70-90 GB/s), so every bulk input is host-packed such
that each DMA has >=4KB contiguous per partition: xT in 2/4-c-tile chunks
(4-8KB runs), V-weights in quads of [128,512] tiles (4KB runs), Q/K weights
per (q/k, head) with both contraction halves fused (4KB runs). x rides the
sync queue, weights ride scalar; a short chain of dummy matmuls on a memset
region warms the PE clock (HAM un-throttle) while the first tiles fly.

Softmax skips the max-subtraction (scores are ~N(0,1); exp is safely in fp16
range). Exp tiles are summed on the DVE and one ones-matrix matmul per
q-chunk then yields the softmax denominator already broadcast across
partitions (a gpsimd partition_all_reduce was tried instead: at ~2.1us/op it
backpressures the chunk pipeline -- keep it on the PE). Causal structure is
exploited at q-chunk=256 granularity (k-tiles 0..2jc+1 per chunk) with the
mask applied as a DVE multiply against precomputed 0/1 tiles. Q/K
projections run as four half-chains interleaved with the attention chunks
that consume them. Per-head attention outputs and a row-contiguous copy of
w_proj stay resident in SBUF, so the output projection starts with
everything on-chip. The first out-proj accumulators borrow the idle Q/K
PSUM banks and the last output tile drains in 256-col chunks to shorten the
tail.
"""

import math

import numpy as np

B, T, D = 8, 1024, 2048
H = 16
DH = D // H  # 128
NCT = D // 128  # 16 c-tiles
QC = 256  # q-chunk for causal attention
NQC = T // QC  # 4
SCALE = 1.0 / math.sqrt(DH)
N_CORES = 8

_CACHE = {}


def _build():
    import concourse.bacc as bacc
    import concourse.mybir as mybir
    import concourse.tile as tile

    f32 = mybir.dt.float32
    f16 = mybir.dt.float16
    Exp = mybir.ActivationFunctionType.Exp

    nc = bacc.Bacc(None, target_bir_lowering=False)

    # x.T partition-packed: [p, ct*1024 + t] = x[t, ct*128 + p]
    xT = nc.declare_dram_parameter("xT", [128, NCT * T], f16, isOutput=False)
    # V weights partition-packed in (fc, quad) blocks: column
    # fc*8192 + ct*512 + j of partition p = w_qkv[ct*128 + p, 2D + fc*512 + j]
    w_vp = nc.declare_dram_parameter("w_vp", [128, 4 * NCT * 512], f16, isOutput=False)
    b_v = nc.declare_dram_parameter("b_v", [D], f16, isOutput=False)
    w_proj = nc.declare_dram_parameter("w_proj", [D, D], f16, isOutput=False)
    # biases host-packed p-major: col n<48 = b_qkv[n*128+p], col 48+n = b_proj[n*128+p]
    bias_pk = nc.declare_dram_parameter("bias_pk", [128, 64], f32, isOutput=False)
    # causal masks (keep where k <= q) for the two diagonal k-tiles, plus a
    # ones block for the softmax-denominator matmul
    consts = nc.declare_dram_parameter(
        "consts", [128, 2 * QC + 128], f16, isOutput=False
    )
    # Q/K weights partition-packed per (s, h): [p, (s*H+h)*2048 + hf*1024 +
    # n*128 + f] = w_qkv[hf*1024 + n*128 + p, s*D + h*128 + f]
    w_qkp = nc.declare_dram_parameter("w_qkp", [128, 2 * H * 2048], f16, isOutput=False)
    outT = nc.declare_dram_parameter("outT", [D, T], f32, isOutput=True)

    with tile.TileContext(nc) as tc:
        with (
            tc.tile_pool(name="xbig", bufs=1) as pool_xbig,
            tc.tile_pool(name="vbig", bufs=1) as pool_vbig,
            tc.tile_pool(name="aobig", bufs=1) as pool_aobig,
            tc.tile_pool(name="qk", bufs=4) as pool_qk,
            tc.tile_pool(name="e", bufs=4) as pool_e,
            tc.tile_pool(name="esum", bufs=2) as pool_esum,
            tc.tile_pool(name="wq", bufs=3) as pool_wq,
            tc.tile_pool(name="wbig", bufs=2) as pool_wbig,
            tc.tile_pool(name="wproj", bufs=2) as pool_wproj,
            tc.tile_pool(name="outp", bufs=2) as pool_out,
            tc.tile_pool(name="den", bufs=2) as pool_den,
            tc.tile_pool(name="misc", bufs=1) as pool_misc,
        ):
            pool_qa_cm = tc.tile_pool(name="qaps", bufs=2, space="PSUM")
            pool_qa = pool_qa_cm.__enter__()

            # ---- PE warmup: a memset region + 6 dummy N=512 matmuls issued
            # before any data lands releases the HAM clock throttle (~3.4us
            # of sustained PE activity) while the first DMAs are in flight,
            # so the first real matmuls run at 2.4 GHz instead of 1.2. The
            # scratch operand borrows V_sb (first written ~30us in). ----
            V_sb = pool_vbig.tile([128, T // 128, D], f16, tag="vbig")
            warm_sb = V_sb[:, 0, 0:512]
            nc.gpsimd.memset(warm_sb, 0.0)
            warm_ps = pool_qa.tile([128, 512], f32, name="warm_ps", tag="qa")
            for _ in range(12):
                nc.tensor.matmul(
                    warm_ps[:], warm_sb[:, 0:128], warm_sb, start=True, stop=True
                )

            # ---- load x.T resident: [128, 16, 1024], one slot per c-tile.
            # Chunked 2/4-tile DMAs (4-8KB contiguous per partition) run at
            # byte rate on the sync queue; fc=0 V-weight quads ride scalar. ----
            xT_all = pool_xbig.tile([128, NCT, T], f16, tag="xbig")
            xT_t = [xT_all[:, ct, :] for ct in range(NCT)]

            def dma_x(c0, c1, eng=None):
                (eng or nc.sync).dma_start(
                    xT_all[:, c0:c1, :], xT[:, c0 * T : c1 * T]
                )

            def dma_wq(fc, q, eng=None, split=False):
                # one quad = c-tiles 4q..4q+3 of feature chunk fc
                w_t = pool_wq.tile([128, 4, 512], f16, name="w_t", tag="wq")
                base = fc * 8192 + q * 2048
                eng = eng or nc.scalar
                if split:
                    # two half-quad DMAs so the first pair lands sooner
                    eng.dma_start(w_t[:, 0:2, :], w_vp[:, base : base + 1024])
                    eng.dma_start(w_t[:, 2:4, :], w_vp[:, base + 1024 : base + 2048])
                else:
                    eng.dma_start(w_t[:], w_vp[:, base : base + 2048])
                return w_t

            # Runs are capped at 4KB per partition: anything bigger (the
            # 8KB-run variant) trips a chip-wide ~0.83x power downclock (P0)
            # that sticks for the whole kernel -- see the v3 trace.
            # Earliest-deadline order on two ~115GB/s queues: ALL x on sync
            # (smallest pieces first so the first chain starts ~11us in),
            # ALL weights + smalls on scalar. The gpsimd software DGE
            # measured <~13GB/s -- useless for bulk.
            dma_x(0, 1)
            wq0 = dma_wq(0, 0, split=True)
            dma_x(1, 2)
            wq1 = dma_wq(0, 1)
            dma_x(2, 4)
            dma_x(4, 6)
            dma_x(6, 8)
            dma_x(8, 10)
            dma_x(10, 12)
            wq2 = dma_wq(0, 2)
            wq3 = dma_wq(0, 3)
            # the x tail rides scalar (3MB per queue, balanced): sync alone
            # would run ~4us behind consumption by c-tile 12, and that
            # stall also re-throttles the HAM clock
            dma_x(12, 14, nc.scalar)
            dma_x(14, 16, nc.scalar)

            # ---- constants / biases: contiguous host-packed, cheap ----
            cpack = pool_misc.tile([128, 2 * QC + 128], f16, tag="cpack")
            nc.scalar.dma_start(cpack[:], consts[:])
            masks = [cpack[:, 0:QC], cpack[:, QC : 2 * QC]]
            ones_blk = cpack[:, 2 * QC : 2 * QC + 128]
            bias_sb = pool_misc.tile([128, 64], f32, tag="biaspk")
            nc.scalar.dma_start(bias_sb[:], bias_pk[:])
            bqkv_sb = bias_sb[:, 0:48]
            bproj_sb = bias_sb[:, 48:64]
            # V-bias broadcast to all partitions once on gpsimd (DMA lands
            # in partition 0, broadcast fills the rest in place); the V
            # PSUM->SBUF copy then fuses the add on the DVE
            bv_bcast = pool_misc.tile([128, D], f16, tag="bv_bcast")
            nc.scalar.dma_start(
                bv_bcast[0:1, :], b_v[:].rearrange("(o f) -> o f", o=1)
            )
            nc.gpsimd.partition_broadcast(bv_bcast[:, :], bv_bcast[0:1, :])

            # ---- phase 1: V for all heads, token-major [128, 8, 2048].
            # Six PSUM banks (token tiles in a 6+2 sub-pass split, weights
            # stay resident across both) so phase 2's Q/K accumulators are
            # pre-allocated and never wait on the phase-1 drain. ----
            with tc.tile_pool(name="p1psum", bufs=6, space="PSUM") as pool_p1:
                for fc in range(D // 512):
                    if fc == 0:
                        w_qs = [wq0, wq1, wq2, wq3]
                    else:
                        w_qs = [dma_wq(fc, q) for q in range(4)]
                    # token tiles 0-5 use the phase-1 banks; tiles 6-7
                    # borrow the (idle) phase-2 Q/K accumulator banks
                    ps_v = [
                        pool_p1.tile([128, 512], f32, name="vps", tag="vps")
                        for _ in range(6)
                    ] + [
                        pool_qa.tile([128, 512], f32, name="vps_qa", tag="qa")
                        for _ in range(2)
                    ]
                    for ct in range(NCT):
                        # tt order matches the drain order below, so the
                        # next fc's first matmul waits only on the first
                        # drained bank instead of the third
                        for tt in (6, 7, 0, 1, 2, 3, 4, 5):
                            nc.tensor.matmul(
                                ps_v[tt][:],
                                xT_t[ct][:, tt * 128 : (tt + 1) * 128],
                                w_qs[ct // 4][:, ct % 4, :],
                                start=(ct == 0),
                                stop=(ct == NCT - 1),
                            )
                    # tiles 6-7 first: they hold the borrowed Q/K banks,
                    # so the first phase-2 chain isn't stuck behind the
                    # whole serial DVE copy drain
                    for tt in (6, 7, 0, 1, 2, 3, 4, 5):
                        # fused += b_v during the PSUM->SBUF copy
                        nc.vector.tensor_add(
                            V_sb[:, tt, fc * 512 : (fc + 1) * 512],
                            ps_v[tt][:],
                            bv_bcast[:, fc * 512 : (fc + 1) * 512],
                        )

            # ---- phase 2: per-head attention. Q/K projections run as four
            # half-chains (Q-half then K-half, bias-adds overlapping the
            # next chain); attention q-chunks 0-1 only need the first halves
            # of Q and K, so they interleave between the half-chains and the
            # PE never waits on a DVE bias-add. w_proj is staged into SBUF
            # row-contiguous (one c-tile per head) so phase 3 starts with
            # all weights resident. ----
            def emit_qk_weights(h, eng=None):
                eng = eng or nc.sync
                w_hs = {}
                for si, s in enumerate(("q", "k")):
                    w_t = pool_wbig.tile(
                        [128, 2, NCT // 2, 128], f16, name="w_t", tag="wbig", bufs=2
                    )
                    base = (si * H + h) * 2048
                    eng.dma_start(w_t[:], w_qkp[:, base : base + 2048])
                    w_hs[s] = [w_t[:, 0], w_t[:, 1]]
                return w_hs

            def emit_qk_half(h, s, w_halves, sb, jc):
                btile = h if s == "q" else NCT + h
                ps = pool_qa.tile([128, 512], f32, name="qkps", tag="qa")
                for ct in range(NCT):
                    nc.tensor.matmul(
                        ps[:],
                        w_halves[ct // 8][:, ct % 8, :],
                        xT_t[ct][:, jc * 512 : (jc + 1) * 512],
                        start=(ct == 0),
                        stop=(ct == NCT - 1),
                    )
                nc.vector.tensor_scalar_add(
                    sb[:, jc * 512 : (jc + 1) * 512],
                    ps[:],
                    bqkv_sb[:, btile : btile + 1],
                )

            def emit_attn_chunks(h, qk, ao_t, jcs):
                # causal attention, scores transposed [k, q],
                # q-chunks of 256 (k-tiles 0..2jc+1; rest masked)
                for jc in jcs:
                    nk = 2 * jc + 2
                    ps_y = pool_y.tile([128, QC], f32, tag="y")
                    e_sum = pool_esum.tile([128, QC], f16, tag="esum", bufs=3)
                    # k-tiles 0..nk-2 run full-width; the final diagonal
                    # k-tile's first 128 q-columns are fully masked, so it
                    # runs at half width into the upper half of the chunk
                    for ki in range(nk - 1):
                        ps_s = pool_s.tile([128, QC], f32, tag="mm256")
                        nc.tensor.matmul(
                            ps_s[:],
                            qk["k"][:, ki * 128 : (ki + 1) * 128],
                            qk["q"][:, jc * QC : (jc + 1) * QC],
                            start=True,
                            stop=True,
                        )
                        # exp of the first k-tile lands directly in e_sum
                        e_t = (
                            e_sum
                            if ki == 0
                            else pool_e.tile([128, QC], f16, tag="e", bufs=4)
                        )
                        nc.scalar.activation(e_t[:], ps_s[:], Exp, scale=SCALE)
                        if ki == 2 * jc:
                            # diagonal tile: keep where k <= q
                            nc.vector.tensor_mul(e_t[:], e_t[:], masks[0])
                        nc.tensor.matmul(
                            ps_y[:],
                            V_sb[:, ki, h * 128 : (h + 1) * 128],
                            e_t[:],
                            start=(ki == 0),
                            stop=(ki == nk - 2),
                        )
                        if ki > 0:
                            nc.vector.tensor_add(e_sum[:], e_sum[:], e_t[:])
                    ki = nk - 1
                    ps_s2 = pool_s.tile([128, 128], f32, name="ps_s2", tag="mm256")
                    nc.tensor.matmul(
                        ps_s2[:],
                        qk["k"][:, ki * 128 : (ki + 1) * 128],
                        qk["q"][:, jc * QC + 128 : (jc + 1) * QC],
                        start=True,
                        stop=True,
                    )
                    e_h = pool_e.tile([128, 128], f16, name="e_h", tag="eh", bufs=3)
                    nc.scalar.activation(e_h[:], ps_s2[:], Exp, scale=SCALE)
                    # within this half, keep where (f-128) >= p: the same
                    # triangle as mask0's first 128 columns
                    nc.vector.tensor_mul(e_h[:], e_h[:], masks[0][:, 0:128])
                    nc.tensor.matmul(
                        ps_y[:, 128:QC],
                        V_sb[:, ki, h * 128 : (h + 1) * 128],
                        e_h[:],
                        start=False,
                        stop=True,
                        skip_group_check=True,
                    )
                    nc.vector.tensor_add(
                        e_sum[:, 128:QC], e_sum[:, 128:QC], e_h[:]
                    )
                    # one ones-matrix matmul yields the denominator already
                    # broadcast across partitions: out[m, q] = sum_k e_sum[k, q]
                    ps_db = pool_s.tile([128, QC], f32, name="ps_db", tag="mm256")
                    nc.tensor.matmul(
                        ps_db[:], ones_blk, e_sum[:], start=True, stop=True
                    )
                    # approx reciprocal (~18 bits; denominators are
                    # bounded away from 0 by the diagonal exp term)
                    inv_b = pool_den.tile([128, QC], f32, name="inv_b", tag="invb")
                    nc.vector.reciprocal_approx_fast(out=inv_b[:], in_=ps_db[:])
                    nc.vector.tensor_mul(
                        ao_t[:, jc * QC : (jc + 1) * QC], ps_y[:], inv_b[:]
                    )

            with (
                tc.tile_pool(name="sps", bufs=4, space="PSUM") as pool_s,
                tc.tile_pool(name="yps", bufs=2, space="PSUM") as pool_y,
            ):
                ao_heads = []
                wp_full = []
                for h in range(H):
                    # stage one row-contiguous c-tile of w_proj per head on
                    # the scalar DMA queue (ready before phase 3 starts)
                    wp_t = pool_wproj.tile(
                        [128, D], f16, name="wp_t", tag="wproj", bufs=NCT
                    )
                    nc.scalar.dma_start(
                        wp_t[:], w_proj[h * 128 : (h + 1) * 128, :]
                    )
                    wp_full.append(wp_t)

                    # head 0's weights ride the scalar queue, which drains
                    # long before the sync queue's startup stream
                    w_hs = emit_qk_weights(h, nc.scalar if h == 0 else None)
                    q_sb = pool_qk.tile([128, T], f16, name="q_sb", tag="qk")
                    k_sb = pool_qk.tile([128, T], f16, name="k_sb", tag="qk")
                    qk = {"q": q_sb, "k": k_sb}
                    ao_t = pool_aobig.tile(
                        [128, T], f16, name="ao_t", tag="aobig", bufs=H
                    )
                    # chunks 0-1 read only the first halves of Q and K, so
                    # they run between the half-chains and hide the DVE
                    # bias-add latency
                    emit_qk_half(h, "q", w_hs["q"], q_sb, 0)
                    emit_qk_half(h, "k", w_hs["k"], k_sb, 0)
                    emit_qk_half(h, "q", w_hs["q"], q_sb, 1)
                    emit_attn_chunks(h, qk, ao_t, (0, 1))
                    emit_qk_half(h, "k", w_hs["k"], k_sb, 1)
                    emit_attn_chunks(h, qk, ao_t, (2, 3))
                    ao_heads.append(ao_t)

            # ---- phase 3: output projection, emitted transposed.
            # rhs for c-tile ct is exactly head ct's attention output
            # (f = h*128 + dh); weights and activations are all resident.
            # dt=0's accumulators come from the (already idle) Q/K banks so
            # the first chain isn't gated on the last attention drain; the
            # final dt drains in 256-col chunks to shorten the tail. ----
            with tc.tile_pool(name="p3psum", bufs=4, space="PSUM") as pool_p3:
                for dt in range(D // 128):
                    if dt == 0:
                        ps3 = [
                            pool_qa.tile([128, 512], f32, name="ps3qa", tag="qa")
                            for _ in range(2)
                        ]
                    else:
                        ps3 = [
                            pool_p3.tile([128, 512], f32, name="ps3", tag="mm512")
                            for _ in range(2)
                        ]
                    last = dt == D // 128 - 1

                    def drain(jc, c0, w):
                        o_t = pool_out.tile([128, w], f32, tag="outp", bufs=4)
                        nc.vector.tensor_scalar_add(
                            o_t[:], ps3[jc][:, c0 : c0 + w], bproj_sb[:, dt : dt + 1]
                        )
                        nc.sync.dma_start(
                            outT[
                                dt * 128 : (dt + 1) * 128,
                                jc * 512 + c0 : jc * 512 + c0 + w,
                            ],
                            o_t[:],
                        )

                    if not last:
                        for ct in range(NCT):
                            for jc in range(2):
                                nc.tensor.matmul(
                                    ps3[jc][:],
                                    wp_full[ct][:, dt * 128 : (dt + 1) * 128],
                                    ao_heads[ct][:, jc * 512 : (jc + 1) * 512],
                                    start=(ct == 0),
                                    stop=(ct == NCT - 1),
                                )
                        drain(0, 0, 512)
                        drain(1, 0, 512)
                    else:
                        # jc-sequential chains, jc1 as two N=256 chains on
                        # their own PSUM tiles (a shared tile serializes the
                        # second chain behind the first drain): every drain
                        # except the final 256-col one overlaps later
                        # matmuls, so the post-last-matmul pipe is one short
                        # add->DMA
                        for ct in range(NCT):
                            nc.tensor.matmul(
                                ps3[0][:],
                                wp_full[ct][:, dt * 128 : (dt + 1) * 128],
                                ao_heads[ct][:, 0:512],
                                start=(ct == 0),
                                stop=(ct == NCT - 1),
                            )
                        drain(0, 0, 512)
                        for half in range(2):
                            ps_h = pool_p3.tile(
                                [128, 256], f32, name="ps3h", tag="mm256c", bufs=2
                            )
                            for ct in range(NCT):
                                nc.tensor.matmul(
                                    ps_h[:],
                                    wp_full[ct][:, dt * 128 : (dt + 1) * 128],
                                    ao_heads[ct][
                                        :, 512 + half * 256 : 512 + half * 256 + 256
                                    ],
                                    start=(ct == 0),
                                    stop=(ct == NCT - 1),
                                )
                            o_t = pool_out.tile([128, 256], f32, tag="outp", bufs=4)
                            nc.vector.tensor_scalar_add(
                                o_t[:], ps_h[:], bproj_sb[:, dt : dt + 1]
                            )
                            # the very last piece goes out on the (idle)
                            # scalar queue so it isn't serialized behind the
                            # previous drains' transfers on sync
                            (nc.scalar if half == 1 else nc.sync).dma_start(
                                outT[
                                    dt * 128 : (dt + 1) * 128,
                                    512 + half * 256 : 768 + half * 256,
                                ],
                                o_t[:],
                            )

            pool_qa_cm.__exit__(None, None, None)

    nc.compile()
    return nc


def _get_nc():
    if "nc" not in _CACHE:
        _CACHE["nc"] = _build()
    return _CACHE["nc"]


def kernel(x, w_qkv, b_qkv, w_proj, b_proj, _trace=False, _trace_kwargs=None):
    from concourse.bass_utils import run_bass_kernel_spmd

    x = np.asarray(x, dtype=np.float32)
    w_qkv = np.asarray(w_qkv, dtype=np.float32)
    b_qkv = np.asarray(b_qkv, dtype=np.float32)
    w_proj = np.asarray(w_proj, dtype=np.float32)
    b_proj = np.asarray(b_proj, dtype=np.float32)

    w_qkv16 = np.ascontiguousarray(w_qkv.astype(np.float16))
    w_proj16 = np.ascontiguousarray(w_proj.astype(np.float16))
    b_v16 = np.ascontiguousarray(b_qkv[2 * D : 3 * D].astype(np.float16))

    # V weights partition-packed: [p, (fc, ct, j)] = w_v[ct*128+p, fc*512+j]
    w_vp = np.ascontiguousarray(
        w_qkv16[:, 2 * D : 3 * D]
        .reshape(NCT, 128, 4, 512)
        .transpose(1, 2, 0, 3)
        .reshape(128, 4 * NCT * 512)
    )

    # Q/K weights partition-packed: [p, (s, h, hf, n, f)] =
    # w_qkv[hf*1024 + n*128 + p, s*D + h*128 + f]
    w_qkp = np.ascontiguousarray(
        w_qkv16[:, : 2 * D]
        .reshape(2, 8, 128, 2, H, 128)
        .transpose(2, 3, 4, 0, 1, 5)
        .reshape(128, 2 * H * 2048)
    )

    # biases packed p-major so the device DMA is one contiguous copy
    bias_pk = np.concatenate(
        [
            b_qkv.reshape(3 * D // 128, 128).T,
            b_proj.reshape(D // 128, 128).T,
        ],
        axis=1,
    ).astype(np.float32)
    bias_pk = np.ascontiguousarray(bias_pk)

    # packed constants: causal masks (keep where k<=q) for the two diagonal
    # k-tiles, and a ones block for the denominator matmul
    consts = np.zeros((128, 2 * QC + 128), dtype=np.float16)
    p = np.arange(128)[:, None]
    f = np.arange(QC)[None, :]
    consts[:, 0:QC] = f >= p
    consts[:, QC : 2 * QC] = f >= p + 128
    consts[:, 2 * QC :] = 1.0

    nc = _get_nc()
    in_maps = []
    for i in range(N_CORES):
        xTi = np.ascontiguousarray(
            x[i]
            .astype(np.float16)
            .reshape(T, NCT, 128)
            .transpose(2, 1, 0)
            .reshape(128, NCT * T)
        )
        in_maps.append(
            {
                "xT": xTi,
                "w_vp": w_vp,
                "b_v": b_v16,
                "w_proj": w_proj16,
                "bias_pk": bias_pk,
                "consts": consts,
                "w_qkp": w_qkp,
            }
        )
    res = run_bass_kernel_spmd(
        nc,
        in_maps,
        list(range(N_CORES)),
        trace=_trace,
        **(_trace_kwargs or {}),
    )
    y = np.stack(
        [np.ascontiguousarray(res.results[i]["outT"].T) for i in range(N_CORES)]
    )
    if _trace:
        _CACHE["last_result"] = res
    return y
